# revision 7
# baseline (speedup 1.0000x reference)
"""Trainium2 Bass kernel for nn_AiriaSNN: 3-layer LIF spiking net, 25 steps.

Layout strategy (pure data parallel over 8 cores, batch-sharded):
  - Each core gets B/8 = 131072 batch rows, processed in tiles of 8 "groups"
    x F columns (one group = F consecutive batch rows; 8 groups packed along
    SBUF partitions so every engine pass uses all 128 lanes for layer 1).
  - Feature-major on-chip layout: L1 state [128=8gx16, F], L2 [64=8gx8, F],
    L3 [24=8gx3, F]. Inter-layer matmuls use block-diagonal weights so each
    PE column carries 8 batch elements.
  - Host pre-transposes x into [n_tiles, 48, F] (partition = 6*g + feature)
    and post-reshapes the device output [n_tiles, T, 24, F] to [T, B, 3].
  - Biases: b1 added once into cur1 (exact, matches reference order);
    b2/b3 folded into per-step spike thresholds (n tracks membrane minus
    accumulated bias; spike test uses (n + b*gamma_t) > 1).
"""

import os
import sys

import numpy as np

_REPO = "/opt/trn_rl_repo"
if _REPO not in sys.path:
    sys.path.insert(0, _REPO)

import bass_rust
import concourse.bass as bass
import concourse.mybir as mybir
import concourse.tile as tile
from concourse.bass_utils import run_bass_kernel_spmd

BETA = 0.95
F32 = mybir.dt.float32

N_CORES = 8
GROUPS = 8  # batch groups packed along partitions


# ---------------------------------------------------------------------------
# Workaround: this walrus build rejects >2 sync waits on one CTRL instruction
# (Tile's tail drain carries one wait per logical processor). Spill the waits
# onto standalone single-wait SP nops emitted right after the drain.
def _patched_drain_and_barrier(self, tick_clock, wait_clock):
    drain_inst = self.nc.sync.drain()
    wait_clock.add_sem_waits(
        drain_inst.ins, tile.ScopedClock({None: tick_clock.global_clock})
    )
    si = drain_inst.ins.sync_info
    if si is not None and len(si.on_wait) > 1:
        waits = list(si.on_wait)
        drain_inst.ins.sync_info = bass_rust.SyncInfo(
            on_wait=[], on_update=list(si.on_update)
        )
        for w in waits:
            nop = self.nc.sync.nop()
            nop.ins.sync_info = bass_rust.SyncInfo(on_wait=[w], on_update=[])
    self.nc.all_engine_barrier()
    assert self.sems is not None
    popped = self.nc._tile_sem_poison_stack.pop()
    assert popped is self._sem_poison
    self.nc.clear_and_free_semaphores(list(self.sems.allocated().values()))
    self.nc.all_engine_barrier()


tile.TileContext._drain_and_barrier = _patched_drain_and_barrier


def _split_excess_waits(nc, max_waits=1):
    """Same walrus limitation for ordinary instructions: hoist excess sync
    waits onto same-engine NoOps inserted just before the instruction."""
    for fn in nc.m.functions:
        for bb in fn.blocks:
            insts = bb.instructions
            pending = []
            for idx, inst in enumerate(insts):
                si = inst.sync_info
                if si is None or len(si.on_wait) <= max_waits:
                    continue
                waits = list(si.on_wait)
                keep = waits[-max_waits:]
                extra = waits[:-max_waits]
                nops = []
                for j in range(0, len(extra), max_waits):
                    nops.append(
                        mybir.InstNoOp(
                            name=nc.get_next_instruction_name(),
                            sync_info=mybir.SyncInfo(
                                on_wait=extra[j : j + max_waits], on_update=[]
                            ),
                            bass_nofuse=True,
                            engine=inst.engine,
                        )
                    )
                inst.sync_info = mybir.SyncInfo(
                    on_wait=keep, on_update=list(si.on_update)
                )
                pending.append((idx, nops))
            for idx, nops in reversed(pending):
                for nop in reversed(nops):
                    insts.insert(idx, nop)


# ---------------------------------------------------------------------------


def _blockdiag(w, groups):
    """Block-diagonal lhsT from per-group lhsT block w [k, m] -> [k*g, m*g]."""
    k, m = w.shape
    out = np.zeros((k * groups, m * groups), dtype=w.dtype)
    for g in range(groups):
        out[g * k : (g + 1) * k, g * m : (g + 1) * m] = w
    return out


def _gammas(T):
    # gamma_t = sum_{i=0..t} beta^i in f64, rounded once to f32
    b = np.float64(np.float32(BETA))
    return np.array([(1 - b ** (t + 1)) / (1 - b) for t in range(T)], dtype=np.float64)


class _Built:
    def __init__(self, nc, n_tiles, F, T):
        self.nc = nc
        self.n_tiles = n_tiles
        self.F = F
        self.T = T


_CACHE = {}


def build_nc(T, n_tiles, F):
    key = (T, n_tiles, F)
    if key in _CACHE:
        return _CACHE[key]

    nc = bass.Bass()
    dt = F32
    Alu = mybir.AluOpType
    Act = mybir.ActivationFunctionType

    x_d = nc.dram_tensor("xprep", [n_tiles, 6 * GROUPS, F], dt, kind="ExternalInput")
    w1_d = nc.dram_tensor("w1t", [6 * GROUPS, 16 * GROUPS], dt, kind="ExternalInput")
    b1_d = nc.dram_tensor("b1c", [16 * GROUPS, 1], dt, kind="ExternalInput")
    w2_d = nc.dram_tensor("w2t", [16 * GROUPS, 8 * GROUPS], dt, kind="ExternalInput")
    w3_d = nc.dram_tensor("w3t", [8 * GROUPS, 3 * GROUPS], dt, kind="ExternalInput")
    th2_d = nc.dram_tensor("thr2", [8 * GROUPS, T], dt, kind="ExternalInput")
    th3_d = nc.dram_tensor("thr3", [3 * GROUPS, T], dt, kind="ExternalInput")
    y_d = nc.dram_tensor("y", [n_tiles, T, 3 * GROUPS, F], dt, kind="ExternalOutput")

    with tile.TileContext(nc) as tc:
        with (
            tc.tile_pool(name="const", bufs=1) as cpool,
            tc.tile_pool(name="state", bufs=2) as spool,
            tc.tile_pool(name="spk", bufs=3) as kpool,
            tc.tile_pool(name="io", bufs=3) as iopool,
            tc.tile_pool(name="psum", bufs=2, space="PSUM") as ppool,
        ):
            w1 = cpool.tile([6 * GROUPS, 16 * GROUPS], dt)
            b1 = cpool.tile([16 * GROUPS, 1], dt)
            w2 = cpool.tile([16 * GROUPS, 8 * GROUPS], dt)
            w3 = cpool.tile([8 * GROUPS, 3 * GROUPS], dt)
            th2 = cpool.tile([8 * GROUPS, T], dt)
            th3 = cpool.tile([3 * GROUPS, T], dt)
            nc.sync.dma_start(w1[:], w1_d[:])
            nc.sync.dma_start(b1[:], b1_d[:])
            nc.sync.dma_start(w2[:], w2_d[:])
            nc.sync.dma_start(w3[:], w3_d[:])
            nc.sync.dma_start(th2[:], th2_d[:])
            nc.sync.dma_start(th3[:], th3_d[:])

            for i in range(n_tiles):
                x_t = iopool.tile([6 * GROUPS, F], dt, tag="x")
                nc.sync.dma_start(x_t[:], x_d[i][:])

                pc1 = ppool.tile([16 * GROUPS, F], dt, tag="pc1")
                nc.tensor.matmul(pc1[:], w1[:], x_t[:], start=True, stop=True)
                cur1 = spool.tile([16 * GROUPS, F], dt, tag="cur1")
                # cur1 = psum + b1 (exact reference order: matmul then +bias)
                nc.any.tensor_scalar(cur1[:], pc1[:], b1[:, 0:1], None, Alu.add)

                n1 = spool.tile([16 * GROUPS, F], dt, tag="n1")
                n2 = spool.tile([8 * GROUPS, F], dt, tag="n2")
                n3 = spool.tile([3 * GROUPS, F], dt, tag="n3")

                s1p = s2p = s3p = None
                for t in range(T):
                    # ---- layer 1
                    if t == 0:
                        nc.vector.tensor_copy(n1[:], cur1[:])  # m1_0 = cur1
                    else:
                        nc.vector.scalar_tensor_tensor(
                            n1[:], n1[:], float(np.float32(BETA)), cur1[:],
                            Alu.mult, Alu.add,
                        )
                        nc.vector.scalar_tensor_tensor(
                            n1[:], s1p[:], -1.0, n1[:], Alu.mult, Alu.add,
                        )
                    s1 = kpool.tile([16 * GROUPS, F], dt, tag="s1")
                    nc.any.tensor_scalar(s1[:], n1[:], 1.0, None, Alu.is_gt)

                    # ---- layer 2: cur2(no bias) accumulated in psum
                    p2 = ppool.tile([8 * GROUPS, F], dt, tag="p2")
                    nc.tensor.matmul(p2[:], w2[:], s1[:], start=True, stop=True)
                    if t == 0:
                        nc.vector.tensor_copy(n2[:], p2[:])
                    else:
                        nc.vector.scalar_tensor_tensor(
                            n2[:], n2[:], float(np.float32(BETA)), p2[:],
                            Alu.mult, Alu.add,
                        )
                        nc.vector.scalar_tensor_tensor(
                            n2[:], s2p[:], -1.0, n2[:], Alu.mult, Alu.add,
                        )
                    s2 = kpool.tile([8 * GROUPS, F], dt, tag="s2")
                    nc.any.tensor_scalar(
                        s2[:], n2[:], th2[:, t : t + 1], 1.0, Alu.add, Alu.is_gt
                    )

                    # ---- layer 3
                    p3 = ppool.tile([3 * GROUPS, F], dt, tag="p3")
                    nc.tensor.matmul(p3[:], w3[:], s2[:], start=True, stop=True)
                    if t == 0:
                        nc.vector.tensor_copy(n3[:], p3[:])
                    else:
                        nc.vector.scalar_tensor_tensor(
                            n3[:], n3[:], float(np.float32(BETA)), p3[:],
                            Alu.mult, Alu.add,
                        )
                        nc.vector.scalar_tensor_tensor(
                            n3[:], s3p[:], -1.0, n3[:], Alu.mult, Alu.add,
                        )
                    s3 = kpool.tile([3 * GROUPS, F], dt, tag="s3")
                    nc.any.tensor_scalar(
                        s3[:], n3[:], th3[:, t : t + 1], 1.0, Alu.add, Alu.is_gt
                    )
                    nc.sync.dma_start(y_d[i][t][:], s3[:])

                    s1p, s2p, s3p = s1, s2, s3

    _split_excess_waits(nc)
    built = _Built(nc, n_tiles, F, T)
    _CACHE[key] = built
    return built


def _prep_consts(W1, b1, W2, b2, W3, b3, T):
    g = _gammas(T)
    w1t = _blockdiag(np.ascontiguousarray(W1.T), GROUPS)  # [48, 128]
    w2t = _blockdiag(np.ascontiguousarray(W2.T), GROUPS)  # [128, 64]
    w3t = _blockdiag(np.ascontiguousarray(W3.T), GROUPS)  # [64, 24]
    b1c = np.tile(b1, GROUPS).reshape(-1, 1).astype(np.float32)  # [128, 1]
    th2 = (np.tile(b2, GROUPS)[:, None].astype(np.float64) * g[None, :]).astype(
        np.float32
    )  # [64, T]
    th3 = (np.tile(b3, GROUPS)[:, None].astype(np.float64) * g[None, :]).astype(
        np.float32
    )  # [24, T]
    return dict(w1t=w1t, b1c=b1c, w2t=w2t, w3t=w3t, thr2=th2, thr3=th3)


def kernel(x, W1, b1, W2, b2, W3, b3, num_steps):
    return _run(x, W1, b1, W2, b2, W3, b3, num_steps, trace=False)[0]


def kernel_profiled(x, W1, b1, W2, b2, W3, b3, num_steps):
    """Returns (output, BassKernelResults) with trace enabled."""
    return _run(x, W1, b1, W2, b2, W3, b3, num_steps, trace=True)


def _run(x, W1, b1, W2, b2, W3, b3, num_steps, trace=False):
    x = np.asarray(x)
    T = int(num_steps)
    B = x.shape[0]
    assert B % N_CORES == 0
    Bc = B // N_CORES

    F = 512
    assert Bc % (GROUPS * F) == 0
    n_tiles = Bc // (GROUPS * F)

    built = build_nc(T, n_tiles, F)
    consts = _prep_consts(
        np.asarray(W1), np.asarray(b1), np.asarray(W2), np.asarray(b2),
        np.asarray(W3), np.asarray(b3), T,
    )

    in_maps = []
    for c in range(N_CORES):
        xc = x[c * Bc : (c + 1) * Bc]
        # [n_tiles, 8, F, 6] -> [n_tiles, 8, 6, F] -> [n_tiles, 48, F]
        xp = np.ascontiguousarray(
            xc.reshape(n_tiles, GROUPS, F, 6).transpose(0, 1, 3, 2)
        ).reshape(n_tiles, 6 * GROUPS, F)
        m = {"xprep": xp}
        m.update(consts)
        in_maps.append(m)

    res = run_bass_kernel_spmd(
        built.nc, in_maps, list(range(N_CORES)), trace=trace
    )

    out = np.empty((T, B, 3), dtype=np.float32)
    for c in range(N_CORES):
        y = res.results[c]["y"]  # [n_tiles, T, 24, F]
        y = y.reshape(n_tiles, T, GROUPS, 3, F)
        # -> [T, n_tiles, GROUPS, F, 3] -> [T, Bc, 3]
        out[:, c * Bc : (c + 1) * Bc, :] = y.transpose(1, 0, 2, 4, 3).reshape(T, Bc, 3)
    return out, res


# revision 13
# speedup vs baseline: 2.0145x; 2.0145x over previous
"""Trainium2 Bass kernel for nn_AiriaSNN: 3-layer LIF spiking net, 25 steps.

Layout strategy (pure data parallel over 8 cores, batch-sharded):
  - Each core gets B/8 = 131072 batch rows, processed in tiles of 8 "groups"
    x F columns (one group = F consecutive batch rows; 8 groups packed along
    SBUF partitions so every engine pass uses all 128 lanes for layer 1).
  - Feature-major on-chip layout: L1 state [128=8gx16, F], L2 [64=8gx8, F],
    L3 [24=8gx3, F]. Inter-layer matmuls use block-diagonal weights so each
    PE column carries 8 batch elements.
  - Host pre-transposes x into [n_tiles, 48, F] (partition = 6*g + feature)
    and post-reshapes the device output [n_tiles, T, 24, F] to [T, B, 3].
  - Biases: b1 added once into cur1 (exact, matches reference order);
    b2/b3 folded into per-step spike thresholds (n tracks membrane minus
    accumulated bias; spike test uses (n + b*gamma_t) > 1).
"""

import os
import sys

import numpy as np

_REPO = "/opt/trn_rl_repo"
if _REPO not in sys.path:
    sys.path.insert(0, _REPO)

import bass_rust
import concourse.bass as bass
import concourse.mybir as mybir
import concourse.tile as tile
from concourse.bass_utils import run_bass_kernel_spmd

BETA = 0.95
F32 = mybir.dt.float32

N_CORES = 8
GROUPS = 8  # batch groups packed along partitions


# ---------------------------------------------------------------------------
# Workaround: this walrus build rejects >2 sync waits on one CTRL instruction
# (Tile's tail drain carries one wait per logical processor). Spill the waits
# onto standalone single-wait SP nops emitted right after the drain.
def _patched_drain_and_barrier(self, tick_clock, wait_clock):
    drain_inst = self.nc.sync.drain()
    wait_clock.add_sem_waits(
        drain_inst.ins, tile.ScopedClock({None: tick_clock.global_clock})
    )
    si = drain_inst.ins.sync_info
    if si is not None and len(si.on_wait) > 1:
        waits = list(si.on_wait)
        drain_inst.ins.sync_info = bass_rust.SyncInfo(
            on_wait=[], on_update=list(si.on_update)
        )
        for w in waits:
            nop = self.nc.sync.nop()
            nop.ins.sync_info = bass_rust.SyncInfo(on_wait=[w], on_update=[])
    self.nc.all_engine_barrier()
    assert self.sems is not None
    popped = self.nc._tile_sem_poison_stack.pop()
    assert popped is self._sem_poison
    self.nc.clear_and_free_semaphores(list(self.sems.allocated().values()))
    self.nc.all_engine_barrier()


tile.TileContext._drain_and_barrier = _patched_drain_and_barrier


def _split_excess_waits(nc, max_waits=1):
    """Same walrus limitation for ordinary instructions: hoist excess sync
    waits onto same-engine NoOps inserted just before the instruction."""
    for fn in nc.m.functions:
        for bb in fn.blocks:
            insts = bb.instructions
            pending = []
            for idx, inst in enumerate(insts):
                si = inst.sync_info
                if si is None or len(si.on_wait) <= max_waits:
                    continue
                waits = list(si.on_wait)
                keep = waits[-max_waits:]
                extra = waits[:-max_waits]
                nops = []
                for j in range(0, len(extra), max_waits):
                    nops.append(
                        mybir.InstNoOp(
                            name=nc.get_next_instruction_name(),
                            sync_info=mybir.SyncInfo(
                                on_wait=extra[j : j + max_waits], on_update=[]
                            ),
                            bass_nofuse=True,
                            engine=inst.engine,
                        )
                    )
                inst.sync_info = mybir.SyncInfo(
                    on_wait=keep, on_update=list(si.on_update)
                )
                pending.append((idx, nops))
            for idx, nops in reversed(pending):
                for nop in reversed(nops):
                    insts.insert(idx, nop)


# ---------------------------------------------------------------------------


def _blockdiag(w, groups):
    """Block-diagonal lhsT from per-group lhsT block w [k, m] -> [k*g, m*g]."""
    k, m = w.shape
    out = np.zeros((k * groups, m * groups), dtype=w.dtype)
    for g in range(groups):
        out[g * k : (g + 1) * k, g * m : (g + 1) * m] = w
    return out


def _gammas(T):
    # gamma_t = sum_{i=0..t} beta^i in f64, rounded once to f32
    b = np.float64(np.float32(BETA))
    return np.array([(1 - b ** (t + 1)) / (1 - b) for t in range(T)], dtype=np.float64)


class _Built:
    def __init__(self, nc, n_tiles, F, T):
        self.nc = nc
        self.n_tiles = n_tiles
        self.F = F
        self.T = T


_CACHE = {}


def build_nc(T, n_tiles, F):
    """v2: processes tiles in groups of 4 ("quad" = 2 pairs of 2 tiles).

    - L1 state per pair merged to [128, 2F] (cur1/n1/s1).
    - L2 pair-packed: psum2/n2/s2 [128, F] (partitions = 2 tiles x 64).
    - L3 quad-packed: psum3/n3/s3 [128, F] (partitions 0:48 pair A, 64:112
      pair B, rest zero-padded).
    - W2/W3 matmuls: bf16 3-chunk decomposition (exact products since
      spikes are 0/1 and chunk weights are exact bf16 values).
    - s1 spike compare on ACT as Relu(Sign(n1 - 1)); s2/s3 compares on DVE.
    """
    key = (T, n_tiles, F)
    if key in _CACHE:
        return _CACHE[key]
    assert n_tiles % 4 == 0
    n_quads = n_tiles // 4

    nc = bass.Bass()
    dt = F32
    bf = mybir.dt.bfloat16
    Alu = mybir.AluOpType
    Act = mybir.ActivationFunctionType
    BETAf = float(np.float32(BETA))

    x_d = nc.dram_tensor("xprep", [n_tiles, 6 * GROUPS, F], dt, kind="ExternalInput")
    w1_d = nc.dram_tensor("w1t", [6 * GROUPS, 16 * GROUPS], dt, kind="ExternalInput")
    b1_d = nc.dram_tensor("b1c", [16 * GROUPS, 1], dt, kind="ExternalInput")
    # 3 bf16 chunks each, blockdiag
    w2_d = nc.dram_tensor("w2c", [16 * GROUPS, 3 * 64], bf, kind="ExternalInput")
    w3_d = nc.dram_tensor("w3c", [16 * 8, 3 * 64], bf, kind="ExternalInput")
    th2_d = nc.dram_tensor("thr2", [128, T], dt, kind="ExternalInput")
    th3_d = nc.dram_tensor("thr3", [128, T], dt, kind="ExternalInput")
    y_d = nc.dram_tensor("y", [n_tiles // 2, T, 48, F], dt, kind="ExternalOutput")

    with tile.TileContext(nc) as tc:
        with (
            tc.tile_pool(name="const", bufs=1) as cpool,
            tc.tile_pool(name="state", bufs=2) as spool,
            tc.tile_pool(name="spk", bufs=3) as kpool,
            tc.tile_pool(name="io", bufs=3) as iopool,
            tc.tile_pool(name="psum", bufs=2, space="PSUM") as ppool,
        ):
            w1 = cpool.tile([6 * GROUPS, 16 * GROUPS], dt)
            b1 = cpool.tile([16 * GROUPS, 1], dt)
            w2 = cpool.tile([16 * GROUPS, 3 * 64], bf)
            w3 = cpool.tile([16 * 8, 3 * 64], bf)
            th2 = cpool.tile([128, T], dt)
            th3 = cpool.tile([128, T], dt)
            nc.sync.dma_start(w1[:], w1_d[:])
            nc.sync.dma_start(b1[:], b1_d[:])
            nc.sync.dma_start(w2[:], w2_d[:])
            nc.sync.dma_start(w3[:], w3_d[:])
            nc.sync.dma_start(th2[:], th2_d[:])
            nc.sync.dma_start(th3[:], th3_d[:])
            negone = cpool.tile([128, 1], dt)
            nc.gpsimd.memset(negone[:], -1.0)

            for q in range(n_quads):
                # ---- per-quad setup: x load + cur1 for 4 tiles (2 pairs)
                cur1s, n1s, s1ps = [], [], []
                for p in range(2):  # pair within quad
                    cur1 = spool.tile([128, 2 * F], dt, tag=f"cur1_{p}")
                    n1 = spool.tile([128, 2 * F], dt, tag=f"n1_{p}")
                    for h in range(2):  # tile within pair
                        ti = q * 4 + p * 2 + h
                        x_t = iopool.tile([6 * GROUPS, F], dt, tag="x")
                        nc.sync.dma_start(x_t[:], x_d[ti][:])
                        pc1 = ppool.tile([16 * GROUPS, F], dt, tag="pc1")
                        nc.tensor.matmul(pc1[:], w1[:], x_t[:], start=True, stop=True)
                        # cur1 half = psum + b1 (matches reference order)
                        nc.any.tensor_scalar(
                            cur1[:, h * F : (h + 1) * F], pc1[:], b1[:, 0:1], None,
                            Alu.add,
                        )
                    cur1s.append(cur1)
                    n1s.append(n1)
                    s1ps.append(None)

                n2s = [spool.tile([128, F], dt, name=f"n2_{p}", tag=f"n2_{p}") for p in range(2)]
                s2ps = [None, None]
                n3 = spool.tile([128, F], dt, tag="n3")
                s3p = None

                for t in range(T):
                    s1s, s2s = [], []
                    for p in range(2):
                        # ---- layer 1 (pair-merged [128, 2F])
                        cur1, n1, s1p = cur1s[p], n1s[p], s1ps[p]
                        if t == 0:
                            nc.vector.tensor_copy(n1[:], cur1[:])
                        else:
                            nc.vector.scalar_tensor_tensor(
                                n1[:], n1[:], BETAf, cur1[:], Alu.mult, Alu.add
                            )
                            nc.vector.scalar_tensor_tensor(
                                n1[:], s1p[:], -1.0, n1[:], Alu.mult, Alu.add
                            )
                        # spike: Relu(Sign(n1 - 1)) on ACT, bf16 out
                        sg = kpool.tile([128, 2 * F], bf, tag=f"sg_{p}")
                        nc.scalar.activation(sg[:], n1[:], Act.Sign, bias=negone[:, 0:1])
                        s1 = kpool.tile([128, 2 * F], bf, tag=f"s1_{p}")
                        nc.scalar.activation(s1[:], sg[:], Act.Relu)
                        s1s.append(s1)

                        # ---- layer 2 pair-packed psum [128, F]
                        p2 = ppool.tile([128, F], dt, tag=f"p2_{p}")
                        for h in range(2):
                            rhs = s1[:, h * F : (h + 1) * F]
                            for c in range(3):
                                nc.tensor.matmul(
                                    p2[h * 64 : (h + 1) * 64, :],
                                    w2[:, c * 64 : (c + 1) * 64],
                                    rhs,
                                    start=(c == 0),
                                    stop=(c == 2),
                                    tile_position=(0, h * 64),
                                )
                        n2, s2p = n2s[p], s2ps[p]
                        if t == 0:
                            nc.vector.tensor_copy(n2[:], p2[:])
                        else:
                            nc.vector.scalar_tensor_tensor(
                                n2[:], n2[:], BETAf, p2[:], Alu.mult, Alu.add
                            )
                            nc.vector.scalar_tensor_tensor(
                                n2[:], s2p[:], -1.0, n2[:], Alu.mult, Alu.add
                            )
                        s2 = kpool.tile([128, F], bf, tag=f"s2_{p}")
                        nc.vector.tensor_scalar(
                            s2[:], n2[:], th2[:, t : t + 1], 1.0, Alu.add, Alu.is_gt
                        )
                        s2s.append(s2)

                    # ---- layer 3 quad-packed psum [128, F]
                    p3 = ppool.tile([128, F], dt, tag="p3")
                    for p in range(2):
                        for c in range(3):
                            nc.tensor.matmul(
                                p3[p * 64 : p * 64 + 64, :],
                                w3[:, c * 64 : (c + 1) * 64],
                                s2s[p][:],
                                start=(c == 0),
                                stop=(c == 2),
                                tile_position=(0, p * 64),
                            )
                    if t == 0:
                        nc.vector.tensor_copy(n3[:], p3[:])
                    else:
                        nc.vector.scalar_tensor_tensor(
                            n3[:], n3[:], BETAf, p3[:], Alu.mult, Alu.add
                        )
                        nc.vector.scalar_tensor_tensor(
                            n3[:], s3p[:], -1.0, n3[:], Alu.mult, Alu.add
                        )
                    s3 = kpool.tile([128, F], dt, tag="s3")
                    nc.vector.tensor_scalar(
                        s3[:], n3[:], th3[:, t : t + 1], 1.0, Alu.add, Alu.is_gt
                    )
                    for p in range(2):
                        nc.sync.dma_start(
                            y_d[q * 2 + p][t][:], s3[p * 64 : p * 64 + 48, :]
                        )
                    s1ps = s1s
                    s2ps = s2s
                    s3p = s3

    _split_excess_waits(nc)
    built = _Built(nc, n_tiles, F, T)
    _CACHE[key] = built
    return built


def _bf16_chunks(w):
    import ml_dtypes

    w = w.astype(np.float32)
    hi = w.astype(ml_dtypes.bfloat16)
    r1 = w - hi.astype(np.float32)
    lo = r1.astype(ml_dtypes.bfloat16)
    r2 = r1 - lo.astype(np.float32)
    lo2 = r2.astype(ml_dtypes.bfloat16)
    return [hi, lo, lo2]


def _prep_consts(W1, b1, W2, b2, W3, b3, T):
    import ml_dtypes

    g = _gammas(T)
    w1t = _blockdiag(np.ascontiguousarray(W1.T), GROUPS)  # [48, 128]
    b1c = np.tile(b1, GROUPS).reshape(-1, 1).astype(np.float32)  # [128, 1]

    # W2: 3 bf16 chunks of blockdiag(W2.T x 8 groups) -> [128, 3*64]
    w2c = np.zeros((128, 3 * 64), dtype=ml_dtypes.bfloat16)
    for c, ch in enumerate(_bf16_chunks(np.ascontiguousarray(W2.T))):
        w2c[:, c * 64 : (c + 1) * 64] = _blockdiag(ch, GROUPS)
    # W3: 3 bf16 chunks of blockdiag(W3.T x 16 groups) [128, 48] pad-> 64
    w3c = np.zeros((128, 3 * 64), dtype=ml_dtypes.bfloat16)
    for c, ch in enumerate(_bf16_chunks(np.ascontiguousarray(W3.T))):
        w3c[:, c * 64 : c * 64 + 48] = _blockdiag(ch, 16)

    # thresholds [128, T]: 1 - accumulated bias, folded as (n + b*gamma > 1)
    o2 = np.tile(b2, 16)  # p -> b2[(p % 64) % 8] over [128]
    th2 = (o2[:, None].astype(np.float64) * g[None, :]).astype(np.float32)
    th3 = np.zeros((128, T), dtype=np.float32)
    for p in range(128):
        r = p % 64
        if r < 48:
            th3[p] = (np.float64(b3[r % 3]) * g).astype(np.float32)
    return dict(w1t=w1t, b1c=b1c, w2c=w2c, w3c=w3c, thr2=th2, thr3=th3)


def kernel(x, W1, b1, W2, b2, W3, b3, num_steps):
    return _run(x, W1, b1, W2, b2, W3, b3, num_steps, trace=False)[0]


def kernel_profiled(x, W1, b1, W2, b2, W3, b3, num_steps):
    """Returns (output, BassKernelResults) with trace enabled."""
    return _run(x, W1, b1, W2, b2, W3, b3, num_steps, trace=True)


def _run(x, W1, b1, W2, b2, W3, b3, num_steps, trace=False):
    x = np.asarray(x)
    T = int(num_steps)
    B = x.shape[0]
    assert B % N_CORES == 0
    Bc = B // N_CORES

    F = 512
    assert Bc % (GROUPS * F) == 0
    n_tiles = Bc // (GROUPS * F)

    built = build_nc(T, n_tiles, F)
    consts = _prep_consts(
        np.asarray(W1), np.asarray(b1), np.asarray(W2), np.asarray(b2),
        np.asarray(W3), np.asarray(b3), T,
    )

    in_maps = []
    for c in range(N_CORES):
        xc = x[c * Bc : (c + 1) * Bc]
        # [n_tiles, 8, F, 6] -> [n_tiles, 8, 6, F] -> [n_tiles, 48, F]
        xp = np.ascontiguousarray(
            xc.reshape(n_tiles, GROUPS, F, 6).transpose(0, 1, 3, 2)
        ).reshape(n_tiles, 6 * GROUPS, F)
        m = {"xprep": xp}
        m.update(consts)
        in_maps.append(m)

    res = run_bass_kernel_spmd(
        built.nc, in_maps, list(range(N_CORES)), trace=trace
    )

    out = np.empty((T, B, 3), dtype=np.float32)
    n_pairs = n_tiles // 2
    for c in range(N_CORES):
        y = res.results[c]["y"]  # [n_pairs, T, 48, F]
        y = y.reshape(n_pairs, T, 16, 3, F)
        # b = pair*16F + k*F + j  -> [T, n_pairs, 16, F, 3] -> [T, Bc, 3]
        out[:, c * Bc : (c + 1) * Bc, :] = y.transpose(1, 0, 2, 4, 3).reshape(T, Bc, 3)
    return out, res
